# revision 8
# baseline (speedup 1.0000x reference)
"""Trainium2 Bass kernel for nn_HC2STARModel (partitioned-norm + center/domain MLPs).

Strategy:
  - Host sorts rows by domain; 2 cores per domain (8 cores, 4 domains), so each
    core runs ONE domain's MLP (4x less compute than the reference's
    all-domains-then-gather).
  - Feature-major ("transposed") activations on device: x is shipped as xT
    (128, 8, S) so every layer is a chain of PE matmuls with K on partitions.
  - LayerNorm is folded into the L1 matmul: per-column mean/std are streamed as
    extra K-rows of the moving operand (stationary rows: -colsum(W1'), b1').
    Since relu(s*z) = s*relu(z) for s>0, the 1/std scale is deferred through
    both hidden layers and applied once at the 128-wide fusion point.
  - pn_w/pn_b (domain affine) folded into W1/b1 on host. The aux head depends
    only on domain_id -> folded into the final bias on host.
"""
import os
import sys

sys.path.insert(0, "/opt/trn_rl_repo")

import numpy as np
import ml_dtypes

BF16 = ml_dtypes.bfloat16

B, D_IN = 16384, 1024
N_DOM = 4
H1, H2, H3, FH = 512, 256, 128, 64
EPS = 1e-5
P = 128
NT = 512  # batch-tile (moving free dim) size

_cache = {}
LAST_RESULTS = None  # stash for test harness profiling


def _build(S):
    from concourse import bass, bacc, tile
    import concourse.mybir as mybir

    dt = mybir.dt
    AF = mybir.ActivationFunctionType
    Alu = mybir.AluOpType

    nc = bacc.Bacc("TRN2", target_bir_lowering=False, debug=False)

    xT = nc.declare_dram_parameter("xT", [P, 8, S], dt.bfloat16, isOutput=False)
    w1 = nc.declare_dram_parameter("w1", [P, 8, 1024], dt.bfloat16, isOutput=False)
    w2c = nc.declare_dram_parameter("w2c", [P, 4, H2], dt.bfloat16, isOutput=False)
    w2d = nc.declare_dram_parameter("w2d", [P, 4, H2], dt.bfloat16, isOutput=False)
    w3c = nc.declare_dram_parameter("w3c", [P, 2, H3], dt.bfloat16, isOutput=False)
    w3d = nc.declare_dram_parameter("w3d", [P, 2, H3], dt.bfloat16, isOutput=False)
    fw1 = nc.declare_dram_parameter("fw1", [P, FH], dt.bfloat16, isOutput=False)
    fw2 = nc.declare_dram_parameter("fw2", [FH, 1], dt.bfloat16, isOutput=False)
    brow = nc.declare_dram_parameter("brow", [1, 3072], dt.bfloat16, isOutput=False)
    out = nc.declare_dram_parameter("out", [1, S], dt.float32, isOutput=True)

    sizes = []
    off = 0
    while off < S:
        n = min(NT, S - off)
        sizes.append((off, n))
        off += n

    with tile.TileContext(nc) as tc:
        with (
            tc.tile_pool(name="wp", bufs=1) as wp,
            tc.tile_pool(name="cst", bufs=1) as cst,
            tc.tile_pool(name="xp", bufs=2) as xp,
            tc.tile_pool(name="ap", bufs=2) as ap,
            tc.tile_pool(name="ps_st", bufs=1, space=bass.MemorySpace.PSUM) as ps_st,
            tc.tile_pool(name="ps_l1", bufs=2, space=bass.MemorySpace.PSUM) as ps_l1,
            tc.tile_pool(name="ps_l2", bufs=2, space=bass.MemorySpace.PSUM) as ps_l2,
            tc.tile_pool(name="ps_l3", bufs=1, space=bass.MemorySpace.PSUM) as ps_l3,
            tc.tile_pool(name="ps_hd", bufs=1, space=bass.MemorySpace.PSUM) as ps_hd,
        ):
            w1_sb = wp.tile([P, 8, 1024], dt.bfloat16, tag="w1")
            nc.sync.dma_start(out=w1_sb[:], in_=w1[:])
            w2c_sb = wp.tile([P, 4, H2], dt.bfloat16, tag="w2c")
            nc.sync.dma_start(out=w2c_sb[:], in_=w2c[:])
            w2d_sb = wp.tile([P, 4, H2], dt.bfloat16, tag="w2d")
            nc.sync.dma_start(out=w2d_sb[:], in_=w2d[:])
            w3c_sb = wp.tile([P, 2, H3], dt.bfloat16, tag="w3c")
            nc.sync.dma_start(out=w3c_sb[:], in_=w3c[:])
            w3d_sb = wp.tile([P, 2, H3], dt.bfloat16, tag="w3d")
            nc.sync.dma_start(out=w3d_sb[:], in_=w3d[:])
            fw1_sb = wp.tile([P, FH], dt.bfloat16, tag="fw1")
            nc.sync.dma_start(out=fw1_sb[:], in_=fw1[:])
            fw2_sb = wp.tile([FH, 1], dt.bfloat16, tag="fw2")
            nc.sync.dma_start(out=fw2_sb[:], in_=fw2[:])
            brow_sb = wp.tile([1, 3072], dt.bfloat16, tag="brow")
            nc.sync.dma_start(out=brow_sb[:], in_=brow[:])

            ones_col = cst.tile([P, 1], dt.bfloat16, tag="ones_col")
            nc.vector.memset(ones_col[:], 1.0)
            ones_r128 = cst.tile([1, P], dt.bfloat16, tag="ones_r128")
            nc.vector.memset(ones_r128[:], 1.0)
            ones_rN = cst.tile([1, NT], dt.bfloat16, tag="ones_rN")
            nc.vector.memset(ones_rN[:], 1.0)
            eps_c = cst.tile([1, 1], dt.float32, tag="eps_c")
            nc.vector.memset(eps_c[:], EPS)

            for (col, N) in sizes:
                xt = xp.tile([P, 8, N], dt.bfloat16, tag="xt")
                nc.sync.dma_start(out=xt[:], in_=xT[:, :, col:col + N])
                xsq = xp.tile([P, 8, N], dt.bfloat16, tag="xsq")
                nc.vector.tensor_mul(xsq[:], xt[:], xt[:])

                # per-column sum / sumsq over all 1024 features (PE reduction)
                st_s = ps_st.tile([1, N], dt.float32, tag="sts")
                for c in range(8):
                    nc.tensor.matmul(st_s[:], ones_col[:], xt[:, c, :],
                                     start=(c == 0), stop=(c == 7))
                st_q = ps_st.tile([1, N], dt.float32, tag="stq")
                for c in range(8):
                    nc.tensor.matmul(st_q[:], ones_col[:], xsq[:, c, :],
                                     start=(c == 0), stop=(c == 7))

                # stats rows: mean (bf16), std = sqrt(var+eps) (bf16)
                mean_row = ap.tile([1, N], dt.bfloat16, tag="meanrow")
                nc.scalar.activation(mean_row[:], st_s[:], AF.Copy, scale=1.0 / D_IN)
                msq = ap.tile([1, N], dt.float32, tag="msq")
                nc.scalar.activation(msq[:], mean_row[:], AF.Square)
                ex2 = ap.tile([1, N], dt.float32, tag="ex2")
                nc.scalar.activation(ex2[:], st_q[:], AF.Copy, scale=1.0 / D_IN)
                veps = ap.tile([1, N], dt.float32, tag="veps")
                nc.vector.tensor_sub(veps[:], ex2[:], msq[:])
                std_row = ap.tile([1, N], dt.bfloat16, tag="stdrow")
                nc.scalar.activation(std_row[:], veps[:], AF.Sqrt, bias=eps_c[:])

                # broadcast std to all partitions, invert -> invstd (128, N) f32
                bc_ps = ps_hd.tile([P, N], dt.float32, tag="phd")
                nc.tensor.matmul(bc_ps[:], ones_r128[:], std_row[:],
                                 start=True, stop=True)
                invstd = ap.tile([P, N], dt.float32, tag="invstd")
                nc.vector.reciprocal(invstd[:], bc_ps[:])

                # L1: out-chunks o=0..3 center, 4..7 domain (W1' = pnw-folded
                # [cW1|dW1]); correction rows via K=2 matmul on [mean; std]
                h1 = ap.tile([P, 8, N], dt.bfloat16, tag="h1")
                for o in range(8):
                    p1 = ps_l1.tile([P, N], dt.float32, tag="p1")
                    for c in range(8):
                        nc.tensor.matmul(p1[:], w1_sb[:, c, o * P:(o + 1) * P],
                                         xt[:, c, :], start=(c == 0), stop=False)
                    nc.tensor.matmul(p1[:], brow_sb[0:1, o * P:(o + 1) * P],
                                     mean_row[:], start=False, stop=False)
                    nc.tensor.matmul(p1[:], brow_sb[0:1, 1024 + o * P:1024 + (o + 1) * P],
                                     std_row[:], start=False, stop=True)
                    nc.scalar.activation(h1[:, o, :], p1[:], AF.Relu)

                # L2 center / domain: K = 4 chunks of h1 + std-row bias
                h2c = ap.tile([P, 2, N], dt.bfloat16, tag="h2c")
                h2d = ap.tile([P, 2, N], dt.bfloat16, tag="h2d")
                for (w2_sb, base, boff, h2) in ((w2c_sb, 0, 0, h2c),
                                                (w2d_sb, 4, H2, h2d)):
                    for o in range(2):
                        p2 = ps_l2.tile([P, N], dt.float32, tag="p2")
                        for c in range(4):
                            nc.tensor.matmul(p2[:], w2_sb[:, c, o * P:(o + 1) * P],
                                             h1[:, base + c, :],
                                             start=(c == 0), stop=False)
                        nc.tensor.matmul(p2[:], brow_sb[0:1, 2048 + boff + o * P:2048 + boff + (o + 1) * P],
                                         std_row[:], start=False, stop=True)
                        nc.scalar.activation(h2[:, o, :], p2[:], AF.Relu)

                # L3 domain then center (shared psum slot), fuse with invstd
                p3d = ps_l3.tile([P, N], dt.float32, tag="p3")
                for c in range(2):
                    nc.tensor.matmul(p3d[:], w3d_sb[:, c, :], h2d[:, c, :],
                                     start=(c == 0), stop=False)
                nc.tensor.matmul(p3d[:], brow_sb[0:1, 2688:2688 + H3], std_row[:],
                                 start=False, stop=True)
                d3 = ap.tile([P, N], dt.bfloat16, tag="d3")
                nc.vector.tensor_mul(d3[:], p3d[:], invstd[:])
                t3 = ap.tile([P, N], dt.bfloat16, tag="t3")
                nc.scalar.activation(t3[:], d3[:], AF.Tanh)

                p3c = ps_l3.tile([P, N], dt.float32, tag="p3")
                for c in range(2):
                    nc.tensor.matmul(p3c[:], w3c_sb[:, c, :], h2c[:, c, :],
                                     start=(c == 0), stop=False)
                nc.tensor.matmul(p3c[:], brow_sb[0:1, 2560:2560 + H3], std_row[:],
                                 start=False, stop=True)
                c3 = ap.tile([P, N], dt.bfloat16, tag="c3")
                nc.vector.tensor_mul(c3[:], p3c[:], invstd[:])

                hf = ap.tile([P, N], dt.bfloat16, tag="hf")
                nc.vector.tensor_mul(hf[:], c3[:], t3[:])

                # head: 128 -> 64 (relu) -> 1, bias via ones-row, then sigmoid
                ph = ps_hd.tile([P, N], dt.float32, tag="phd")
                nc.tensor.matmul(ph[0:FH, :], fw1_sb[:], hf[:], start=True, stop=False)
                nc.tensor.matmul(ph[0:FH, :], brow_sb[0:1, 2816:2816 + FH], ones_rN[0:1, 0:N],
                                 start=False, stop=True)
                fh = ap.tile([FH, N], dt.bfloat16, tag="fh")
                nc.scalar.activation(fh[:], ph[0:FH, :], AF.Relu)

                pm = ps_hd.tile([P, N], dt.float32, tag="phd")
                nc.tensor.matmul(pm[0:1, :], fw2_sb[:], fh[:], start=True, stop=False)
                nc.tensor.matmul(pm[0:1, :], brow_sb[0:1, 2880:2881], ones_rN[0:1, 0:N],
                                 start=False, stop=True)
                orow = ap.tile([1, N], dt.float32, tag="orow")
                nc.scalar.activation(orow[:], pm[0:1, :], AF.Sigmoid)
                nc.sync.dma_start(out=out[0:1, col:col + N], in_=orow[:])

    nc.compile()
    return nc


def _prep_core(x_rows, dmn, prm, S):
    """Build the per-core input map for one core handling domain `dmn`."""
    cW1, cb1 = prm["cW1"], prm["cb1"]
    dW1, db1 = prm["dW1"][dmn], prm["db1"][dmn]
    pnw, pnb = prm["pn_w"][dmn], prm["pn_b"][dmn]

    W1cat_raw = np.concatenate([cW1, dW1], axis=1)           # (1024, 1024)
    W1cat = W1cat_raw * pnw[:, None]
    b1 = np.concatenate([cb1, db1]) + pnb @ W1cat_raw         # (1024,)
    colsum1 = W1cat.sum(axis=0)

    de = prm["dom_emb"][dmn]
    aux = np.maximum(de @ prm["aW1"] + prm["ab1"], 0.0) @ prm["aW2"] + prm["ab2"]

    brow = np.zeros((1, 3072), np.float32)
    brow[0, 0:1024] = -colsum1
    brow[0, 1024:2048] = b1
    brow[0, 2048:2048 + H2] = prm["cb2"]
    brow[0, 2048 + H2:2048 + 2 * H2] = prm["db2"][dmn]
    brow[0, 2560:2560 + H3] = prm["cb3"]
    brow[0, 2688:2688 + H3] = prm["db3"][dmn]
    brow[0, 2816:2816 + FH] = prm["fb1"]
    brow[0, 2880] = prm["fb2"][0] + aux[0]

    xc = np.zeros((S, D_IN), np.float32)
    xc[: len(x_rows)] = x_rows
    xTc = np.ascontiguousarray(xc.T.reshape(8, P, S).transpose(1, 0, 2))

    def shp(w, nchunk):  # (K, M) -> (128, K//128, M) SBUF layout
        return np.ascontiguousarray(
            w.reshape(nchunk, P, w.shape[1]).transpose(1, 0, 2)).astype(BF16)

    return {
        "xT": xTc.astype(BF16),
        "w1": shp(W1cat, 8),
        "w2c": shp(prm["cW2"], 4),
        "w2d": shp(prm["dW2"][dmn], 4),
        "w3c": shp(prm["cW3"], 2),
        "w3d": shp(prm["dW3"][dmn], 2),
        "fw1": prm["fW1"].astype(BF16),
        "fw2": prm["fW2"].astype(BF16),
        "brow": brow.astype(BF16),
    }


def kernel(**inputs):
    global LAST_RESULTS
    from concourse.bass_utils import run_bass_kernel_spmd

    prm = {k: np.asarray(v, np.float32) for k, v in inputs.items()
           if k not in ("domain_ids",)}
    x = prm["x"]
    dom = np.asarray(inputs["domain_ids"]).astype(np.int64).reshape(-1)
    in_dtype = np.asarray(inputs["x"]).dtype

    order = np.argsort(dom, kind="stable")
    sorted_dom = dom[order]
    bounds = np.searchsorted(sorted_dom, np.arange(N_DOM + 1))
    core_rows, core_dom = [], []
    for d in range(N_DOM):
        idx = order[bounds[d]:bounds[d + 1]]
        h = (len(idx) + 1) // 2
        core_rows += [idx[:h], idx[h:]]
        core_dom += [d, d]

    S = max(len(r) for r in core_rows)
    S = max(((S + P - 1) // P) * P, P)

    if S not in _cache:
        _cache[S] = _build(S)
    nc = _cache[S]

    in_maps = [_prep_core(x[core_rows[c]], core_dom[c], prm, S)
               for c in range(8)]

    trace = bool(int(os.environ.get("KERNEL_TRACE", "0")))
    res = run_bass_kernel_spmd(nc, in_maps, list(range(8)), trace=trace)
    LAST_RESULTS = res

    out = np.zeros((B, 1), np.float32)
    for c in range(8):
        o = np.asarray(res.results[c]["out"], np.float32).reshape(-1)
        out[core_rows[c], 0] = o[: len(core_rows[c])]
    return out.astype(in_dtype)


# revision 9
# speedup vs baseline: 1.0153x; 1.0153x over previous
"""Trainium2 Bass kernel for nn_HC2STARModel (partitioned-norm + center/domain MLPs).

Strategy:
  - Host sorts rows by domain; 2 cores per domain (8 cores, 4 domains), so each
    core runs ONE domain's MLP (4x less compute than the reference's
    all-domains-then-gather).
  - Feature-major ("transposed") activations on device: x is shipped as xT
    (128, 8, S) so every layer is a chain of PE matmuls with K on partitions.
  - LayerNorm folded into the L1 matmul: the -mean*colsum(W1') correction is an
    extra K=1 row of the moving operand; 1/std is applied at L1 eviction with a
    fused DVE (max(z,0) * invstd) op, so downstream layers are a plain MLP.
  - All downstream biases ride the ACT engine's per-partition bias port (free).
  - pn_w/pn_b folded into W1/b1 on host; the aux head depends only on
    domain_id -> folded into the sigmoid bias on host.
"""
import os
import sys

sys.path.insert(0, "/opt/trn_rl_repo")

import numpy as np
import ml_dtypes

BF16 = ml_dtypes.bfloat16

B, D_IN = 16384, 1024
N_DOM = 4
H1, H2, H3, FH = 512, 256, 128, 64
EPS = 1e-5
P = 128
NT = 512  # batch-tile (moving free dim) size

_cache = {}
LAST_RESULTS = None  # stash for test harness profiling


def _build(S, has_b1):
    from concourse import bass, bacc, tile
    import concourse.mybir as mybir

    dt = mybir.dt
    AF = mybir.ActivationFunctionType
    Alu = mybir.AluOpType

    nc = bacc.Bacc("TRN2", target_bir_lowering=False, debug=False)

    xT = nc.declare_dram_parameter("xT", [P, 8, S], dt.bfloat16, isOutput=False)
    w1 = nc.declare_dram_parameter("w1", [8, P, 8, P], dt.bfloat16, isOutput=False)
    w2c = nc.declare_dram_parameter("w2c", [P, 4, H2], dt.bfloat16, isOutput=False)
    w2d = nc.declare_dram_parameter("w2d", [P, 4, H2], dt.bfloat16, isOutput=False)
    w3c = nc.declare_dram_parameter("w3c", [P, 2, H3], dt.bfloat16, isOutput=False)
    w3d = nc.declare_dram_parameter("w3d", [P, 2, H3], dt.bfloat16, isOutput=False)
    fw1 = nc.declare_dram_parameter("fw1", [P, FH], dt.bfloat16, isOutput=False)
    fw2 = nc.declare_dram_parameter("fw2", [FH, 1], dt.bfloat16, isOutput=False)
    brow = nc.declare_dram_parameter("brow", [1, 2048], dt.bfloat16, isOutput=False)
    bcols = nc.declare_dram_parameter("bcols", [P, 8], dt.float32, isOutput=False)
    out = nc.declare_dram_parameter("out", [1, S], dt.float32, isOutput=True)

    sizes = []
    off = 0
    while off < S:
        n = min(NT, S - off)
        sizes.append((off, n))
        off += n

    with tile.TileContext(nc) as tc:
        with (
            tc.tile_pool(name="wp", bufs=1) as wp,
            tc.tile_pool(name="cst", bufs=1) as cst,
            tc.tile_pool(name="xp", bufs=2) as xp,
            tc.tile_pool(name="ap", bufs=2) as ap,
            tc.tile_pool(name="ps_st", bufs=1, space=bass.MemorySpace.PSUM) as ps_st,
            tc.tile_pool(name="ps_l1", bufs=2, space=bass.MemorySpace.PSUM) as ps_l1,
            tc.tile_pool(name="ps_l2", bufs=2, space=bass.MemorySpace.PSUM) as ps_l2,
            tc.tile_pool(name="ps_l3", bufs=1, space=bass.MemorySpace.PSUM) as ps_l3,
            tc.tile_pool(name="ps_hd", bufs=1, space=bass.MemorySpace.PSUM) as ps_hd,
        ):
            ones_col = cst.tile([P, 1], dt.bfloat16, tag="ones_col")
            nc.vector.memset(ones_col[:], 1.0)
            ones_r128 = cst.tile([1, P], dt.bfloat16, tag="ones_r128")
            nc.vector.memset(ones_r128[:], 1.0)
            eps_c = cst.tile([1, 1], dt.float32, tag="eps_c")
            nc.vector.memset(eps_c[:], EPS)

            # w1 split into 8 out-chunk DMAs so L1 can start early
            w1_sb = wp.tile([P, 8, 8, P], dt.bfloat16, tag="w1")
            for o in range(8):
                nc.sync.dma_start(out=w1_sb[:, o, :, :], in_=w1[o])
            w2c_sb = wp.tile([P, 4, H2], dt.bfloat16, tag="w2c")
            nc.sync.dma_start(out=w2c_sb[:], in_=w2c[:])
            w2d_sb = wp.tile([P, 4, H2], dt.bfloat16, tag="w2d")
            nc.sync.dma_start(out=w2d_sb[:], in_=w2d[:])
            w3c_sb = wp.tile([P, 2, H3], dt.bfloat16, tag="w3c")
            nc.sync.dma_start(out=w3c_sb[:], in_=w3c[:])
            w3d_sb = wp.tile([P, 2, H3], dt.bfloat16, tag="w3d")
            nc.sync.dma_start(out=w3d_sb[:], in_=w3d[:])
            fw1_sb = wp.tile([P, FH], dt.bfloat16, tag="fw1")
            nc.sync.dma_start(out=fw1_sb[:], in_=fw1[:])
            fw2_sb = wp.tile([FH, 1], dt.bfloat16, tag="fw2")
            nc.sync.dma_start(out=fw2_sb[:], in_=fw2[:])
            brow_sb = wp.tile([1, 2048], dt.bfloat16, tag="brow")
            nc.sync.dma_start(out=brow_sb[:], in_=brow[:])
            bcols_sb = wp.tile([P, 8], dt.float32, tag="bcols")
            nc.sync.dma_start(out=bcols_sb[:], in_=bcols[:])

            for (col, N) in sizes:
                xt = xp.tile([P, 8, N], dt.bfloat16, tag="xt")
                nc.sync.dma_start(out=xt[:], in_=xT[:, :, col:col + N])
                xsq = xp.tile([P, 8, N], dt.bfloat16, tag="xsq")
                nc.vector.tensor_mul(xsq[:], xt[:], xt[:])

                # per-column sum / sumsq over all 1024 features (PE reduction)
                st_s = ps_st.tile([1, N], dt.float32, tag="sts")
                for c in range(8):
                    nc.tensor.matmul(st_s[:], ones_col[:], xt[:, c, :],
                                     start=(c == 0), stop=(c == 7))
                st_q = ps_st.tile([1, N], dt.float32, tag="stq")
                for c in range(8):
                    nc.tensor.matmul(st_q[:], ones_col[:], xsq[:, c, :],
                                     start=(c == 0), stop=(c == 7))

                # stats rows: mean (bf16), invstd = 1/sqrt(var+eps) (bf16)
                mean_row = ap.tile([1, N], dt.bfloat16, tag="meanrow")
                nc.scalar.activation(mean_row[:], st_s[:], AF.Copy, scale=1.0 / D_IN)
                msq = ap.tile([1, N], dt.float32, tag="msq")
                nc.vector.tensor_mul(msq[:], mean_row[:], mean_row[:])
                ex2 = ap.tile([1, N], dt.float32, tag="ex2")
                nc.scalar.activation(ex2[:], st_q[:], AF.Copy, scale=1.0 / D_IN)
                veps = ap.tile([1, N], dt.float32, tag="veps")
                nc.vector.tensor_sub(veps[:], ex2[:], msq[:])
                invstd_row = ap.tile([1, N], dt.bfloat16, tag="invrow")
                nc.scalar.activation(invstd_row[:], veps[:], AF.Abs_reciprocal_sqrt,
                                     bias=eps_c[:])
                if has_b1:
                    std_row = ap.tile([1, N], dt.bfloat16, tag="stdrow")
                    nc.scalar.activation(std_row[:], veps[:], AF.Sqrt, bias=eps_c[:])

                # broadcast invstd to all partitions -> (128, N) bf16
                bc_ps = ps_hd.tile([P, N], dt.float32, tag="phd")
                nc.tensor.matmul(bc_ps[:], ones_r128[:], invstd_row[:],
                                 start=True, stop=True)
                invstd = ap.tile([P, N], dt.bfloat16, tag="invstd")
                nc.scalar.activation(invstd[:], bc_ps[:], AF.Copy)

                # L1: out-chunks o=0..3 center, 4..7 domain (W1' = pnw-folded
                # [cW1|dW1]); mean correction via K=1 matmul; eviction fuses
                # relu + invstd scale on DVE
                h1 = ap.tile([P, 8, N], dt.bfloat16, tag="h1")
                for o in range(8):
                    p1 = ps_l1.tile([P, N], dt.float32, tag="p1")
                    for c in range(8):
                        nc.tensor.matmul(p1[:], w1_sb[:, o, c, :],
                                         xt[:, c, :], start=(c == 0), stop=False)
                    last = not has_b1
                    nc.tensor.matmul(p1[:], brow_sb[0:1, o * P:(o + 1) * P],
                                     mean_row[:], start=False, stop=last)
                    if has_b1:
                        nc.tensor.matmul(
                            p1[:], brow_sb[0:1, 1024 + o * P:1024 + (o + 1) * P],
                            std_row[:], start=False, stop=True)
                    nc.vector.scalar_tensor_tensor(h1[:, o, :], p1[:], 0.0,
                                                   invstd[:], Alu.max, Alu.mult)

                # L2 center / domain: plain matmul, bias+relu on ACT eviction
                h2c = ap.tile([P, 2, N], dt.bfloat16, tag="h2c")
                h2d = ap.tile([P, 2, N], dt.bfloat16, tag="h2d")
                for (w2_sb, base, bcol, h2) in ((w2c_sb, 0, 0, h2c),
                                                (w2d_sb, 4, 2, h2d)):
                    for o in range(2):
                        p2 = ps_l2.tile([P, N], dt.float32, tag="p2")
                        for c in range(4):
                            nc.tensor.matmul(p2[:], w2_sb[:, c, o * P:(o + 1) * P],
                                             h1[:, base + c, :],
                                             start=(c == 0), stop=(c == 3))
                        nc.scalar.activation(h2[:, o, :], p2[:], AF.Relu,
                                             bias=bcols_sb[:, bcol + o:bcol + o + 1])

                # L3 domain -> tanh (bias on ACT); L3 center fused into hf
                p3d = ps_l3.tile([P, N], dt.float32, tag="p3")
                for c in range(2):
                    nc.tensor.matmul(p3d[:], w3d_sb[:, c, :], h2d[:, c, :],
                                     start=(c == 0), stop=(c == 1))
                t3 = ap.tile([P, N], dt.bfloat16, tag="t3")
                nc.scalar.activation(t3[:], p3d[:], AF.Tanh,
                                     bias=bcols_sb[:, 5:6])

                p3c = ps_l3.tile([P, N], dt.float32, tag="p3")
                for c in range(2):
                    nc.tensor.matmul(p3c[:], w3c_sb[:, c, :], h2c[:, c, :],
                                     start=(c == 0), stop=(c == 1))
                hf = ap.tile([P, N], dt.bfloat16, tag="hf")
                nc.vector.scalar_tensor_tensor(hf[:], p3c[:], bcols_sb[:, 4:5],
                                               t3[:], Alu.add, Alu.mult)

                # head: 128 -> 64 (relu) -> 1 -> sigmoid; biases on ACT
                ph = ps_hd.tile([P, N], dt.float32, tag="phd")
                nc.tensor.matmul(ph[0:FH, :], fw1_sb[:], hf[:], start=True, stop=True)
                fh = ap.tile([FH, N], dt.bfloat16, tag="fh")
                nc.scalar.activation(fh[:], ph[0:FH, :], AF.Relu,
                                     bias=bcols_sb[0:FH, 6:7])

                pm = ps_hd.tile([P, N], dt.float32, tag="phd")
                nc.tensor.matmul(pm[0:1, :], fw2_sb[:], fh[:], start=True, stop=True)
                orow = ap.tile([1, N], dt.float32, tag="orow")
                nc.scalar.activation(orow[:], pm[0:1, :], AF.Sigmoid,
                                     bias=bcols_sb[0:1, 7:8])
                nc.sync.dma_start(out=out[0:1, col:col + N], in_=orow[:])

    nc.compile()
    return nc


def _prep_core(x_rows, dmn, prm, S):
    """Build the per-core input map for one core handling domain `dmn`."""
    cW1, cb1 = prm["cW1"], prm["cb1"]
    dW1, db1 = prm["dW1"][dmn], prm["db1"][dmn]
    pnw, pnb = prm["pn_w"][dmn], prm["pn_b"][dmn]

    W1cat_raw = np.concatenate([cW1, dW1], axis=1)           # (1024, 1024)
    W1cat = W1cat_raw * pnw[:, None]
    b1 = np.concatenate([cb1, db1]) + pnb @ W1cat_raw         # (1024,)
    colsum1 = W1cat.sum(axis=0)

    de = prm["dom_emb"][dmn]
    aux = np.maximum(de @ prm["aW1"] + prm["ab1"], 0.0) @ prm["aW2"] + prm["ab2"]

    brow = np.zeros((1, 2048), np.float32)
    brow[0, 0:1024] = -colsum1
    brow[0, 1024:2048] = b1

    bcols = np.zeros((P, 8), np.float32)
    bcols[:, 0] = prm["cb2"][:P]
    bcols[:, 1] = prm["cb2"][P:]
    bcols[:, 2] = prm["db2"][dmn][:P]
    bcols[:, 3] = prm["db2"][dmn][P:]
    bcols[:, 4] = prm["cb3"]
    bcols[:, 5] = prm["db3"][dmn]
    bcols[:FH, 6] = prm["fb1"]
    bcols[0, 7] = prm["fb2"][0] + aux[0]

    xc = np.zeros((S, D_IN), np.float32)
    xc[: len(x_rows)] = x_rows
    xTc = np.ascontiguousarray(xc.T.reshape(8, P, S).transpose(1, 0, 2))

    def shp(w, nchunk):  # (K, M) -> (128, K//128, M) SBUF layout
        return np.ascontiguousarray(
            w.reshape(nchunk, P, w.shape[1]).transpose(1, 0, 2)).astype(BF16)

    # w1: (8 out-chunks, 128 p, 8 k-chunks, 128 m)
    w1o = np.ascontiguousarray(
        W1cat.reshape(8, P, 8, P).transpose(2, 1, 0, 3)).astype(BF16)

    return {
        "xT": xTc.astype(BF16),
        "w1": w1o,
        "w2c": shp(prm["cW2"], 4),
        "w2d": shp(prm["dW2"][dmn], 4),
        "w3c": shp(prm["cW3"], 2),
        "w3d": shp(prm["dW3"][dmn], 2),
        "fw1": prm["fW1"].astype(BF16),
        "fw2": prm["fW2"].astype(BF16),
        "brow": brow.astype(BF16),
        "bcols": bcols,
    }, float(np.max(np.abs(b1)))


def kernel(**inputs):
    global LAST_RESULTS
    from concourse.bass_utils import run_bass_kernel_spmd

    prm = {k: np.asarray(v, np.float32) for k, v in inputs.items()
           if k not in ("domain_ids",)}
    x = prm["x"]
    dom = np.asarray(inputs["domain_ids"]).astype(np.int64).reshape(-1)
    in_dtype = np.asarray(inputs["x"]).dtype

    order = np.argsort(dom, kind="stable")
    sorted_dom = dom[order]
    bounds = np.searchsorted(sorted_dom, np.arange(N_DOM + 1))
    core_rows, core_dom = [], []
    for d in range(N_DOM):
        idx = order[bounds[d]:bounds[d + 1]]
        h = (len(idx) + 1) // 2
        core_rows += [idx[:h], idx[h:]]
        core_dom += [d, d]

    S = max(len(r) for r in core_rows)
    S = max(((S + P - 1) // P) * P, P)

    maps_and_b1 = [_prep_core(x[core_rows[c]], core_dom[c], prm, S)
                   for c in range(8)]
    in_maps = [m for m, _ in maps_and_b1]
    has_b1 = any(b > 0.0 for _, b in maps_and_b1)

    key = (S, has_b1)
    if key not in _cache:
        _cache[key] = _build(S, has_b1)
    nc = _cache[key]

    trace = bool(int(os.environ.get("KERNEL_TRACE", "0")))
    res = run_bass_kernel_spmd(nc, in_maps, list(range(8)), trace=trace)
    LAST_RESULTS = res

    out = np.zeros((B, 1), np.float32)
    for c in range(8):
        o = np.asarray(res.results[c]["out"], np.float32).reshape(-1)
        out[core_rows[c], 0] = o[: len(core_rows[c])]
    return out.astype(in_dtype)
